# revision 1
# baseline (speedup 1.0000x reference)
"""Trainium2 Bass kernel for CoEncoderDynamicAttention.

Model (reference):
  q = x @ wq   -> [B,S,NH,HD];  k = x @ wk -> [B,S,NKV,HD];  v = x @ wv
  scores = q k^T / sqrt(HD), masked, softmax over k
  out = (attn @ v) reshaped @ wo        (wo: [NH*HD, 1])

Sharding: 8 cores = (batch b in 0..1) x (kv-group g in 0..3).  Each kv
group owns 1 kv head and GQ=4 q heads.  Since wo has output dim 1, fold
wo into v on the host:  u_h = v_g @ wo_h, so per-core output is
  num_h[q] = sum_k exp(s_h[q,k]) * u_h[k],  den_h[q] = sum_k exp(s_h[q,k])
and the host combines  out[b,q] = sum_{g,h} num_h/den_h.

Device layout (per core):
  xT   [H, S]    hidden[b] transposed (host-prepped)
  wq   [H, 256]  4 q-heads' projection (pair p stored at partitions 0-63/64-127)
  wk   [H, 64]   kv head projection
  wu   [H, 4]    wv_g @ wo_g  (host-folded)
  lm   [128, S/128]  log-mask, exp bias trick: exp(s/8 + lm) == mask * exp(s/8)

On device: QT[d,s] per head pair (row-paired at partitions 0-63 / 64-127),
KT2[d,s] duplicated to both partition halves so scores matmuls for the two
heads of a pair run concurrently in disjoint PE row groups.  Scores are
computed transposed (k on partitions) so exp output feeds the second matmul
as the moving operand with k as the contraction dim.
"""

import numpy as np

import concourse.bass as bass
import concourse.mybir as mybir
import concourse.tile as tile
from concourse.bass_utils import run_bass_kernel_spmd

B, S, H = 2, 2048, 1024
NH, NKV, HD = 16, 4, 64
GQ = NH // NKV          # q heads per kv group
EW = GQ * HD            # per-core q projection width (256)
NCORES = 8
P = 128
F32 = mybir.dt.float32
F32R = mybir.dt.float32r
AF = mybir.ActivationFunctionType


def _split_excess_waits(nc, limit=1):
    """This walrus build only accepts one sync-wait (and update) per
    instruction; hoist extras onto NoOps on the same engine."""
    for f in nc.m.functions:
        for bb in f.blocks:
            new = []
            for inst in bb.instructions:
                si = getattr(inst, "sync_info", None)
                waits = list(si.on_wait) if (si is not None and si.on_wait) else []
                k = 0
                while len(waits) > limit:
                    chunk, waits = waits[:limit], waits[limit:]
                    nop = mybir.InstNoOp(name=f"{inst.name}-ws{k}", ins=[], outs=[])
                    nop.engine = inst.engine
                    nop.sync_info = mybir.SyncInfo(on_wait=chunk, on_update=[])
                    nc.register_instruction(nop)
                    new.append(nop)
                    k += 1
                if k:
                    si.on_wait = waits
                new.append(inst)
                ups = list(si.on_update) if (si is not None and si.on_update) else []
                if len(ups) > limit and type(inst).__name__ not in (
                    "InstDMA", "InstDMACopy", "InstTensorLoad", "InstTensorSave",
                ):
                    si.on_update = ups[:limit]
                    for j, up in enumerate(ups[limit:]):
                        nop = mybir.InstNoOp(name=f"{inst.name}-us{j}", ins=[], outs=[])
                        nop.engine = inst.engine
                        nop.sync_info = mybir.SyncInfo(on_wait=[], on_update=[up])
                        nc.register_instruction(nop)
                        new.append(nop)
            bb.instructions[:] = new


def build_nc(s=S, repeat=1):
    """Build the per-core Bass program.  `s` parametrizes sequence length for
    cheap smoke tests; `repeat` unrolls the whole compute for HW timing."""
    st = s // P             # number of k tiles
    qc_w = min(512, s)      # q chunk width
    nqc = s // qc_w
    ht = H // P             # h (contraction) tiles

    nc = bass.Bass()
    xT = nc.dram_tensor("xT", [H, s], F32R, kind="ExternalInput")
    wq = nc.dram_tensor("wq", [H, EW], F32R, kind="ExternalInput")
    wk = nc.dram_tensor("wk", [H, HD], F32R, kind="ExternalInput")
    wu = nc.dram_tensor("wu", [H, GQ], F32R, kind="ExternalInput")
    lm = nc.dram_tensor("lm", [P, st], F32, kind="ExternalInput")
    uscr = nc.dram_tensor("uscr", [GQ, s], F32)
    out = nc.dram_tensor("out", [GQ, 2, s], F32, kind="ExternalOutput")

    with tile.TileContext(nc) as tc:
        with (
            tc.tile_pool(name="persist", bufs=1) as persist,
            tc.tile_pool(name="pcopy", bufs=4) as pcopy,
            tc.tile_pool(name="exp", bufs=4) as epool,
            tc.tile_pool(name="psum_p", bufs=2, space="PSUM") as psum_p,
            tc.tile_pool(name="psum_s", bufs=2, space="PSUM") as psum_s,
            tc.tile_pool(name="psum_o", bufs=1, space="PSUM") as psum_o,
        ):
            xT_sb = persist.tile([P, ht, s], F32R)
            wq_sb = persist.tile([P, ht, EW], F32R)
            wku_sb = persist.tile([P, ht, HD + GQ], F32R)
            lm_sb = persist.tile([P, st], F32)
            QT = persist.tile([P, 2, s], F32R)     # pair p: partitions 0-63 / 64-127
            KT2 = persist.tile([P, s], F32R)       # KT duplicated to both halves
            MU = persist.tile([P, st, 2 * GQ], F32R)  # per k-tile: [u0,1,u1,1,u2,1,u3,1]
            UTst = persist.tile([P, s], F32)          # rows 64-67: U^T staging

            xTr = xT.rearrange("(t p) s -> p t s", p=P)
            nc.sync.dma_start(out=wku_sb[:, :, 0:HD],
                              in_=wk.rearrange("(t p) e -> p t e", p=P))
            nc.sync.dma_start(out=wku_sb[:, :, HD:HD + GQ],
                              in_=wu.rearrange("(t p) e -> p t e", p=P))
            nc.sync.dma_start(out=lm_sb[:], in_=lm[:, :])
            for q in range(nqc):
                sl = slice(q * qc_w, (q + 1) * qc_w)
                for t in range(ht):
                    nc.sync.dma_start(out=xT_sb[:, t, sl], in_=xTr[:, t, sl])
                if q == 0:
                    nc.sync.dma_start(out=wq_sb[:],
                                      in_=wq.rearrange("(t p) e -> p t e", p=P))
            # 1.0f is exactly representable in f32r; memset via an f32 view
            nc.vector.memset(MU[:, :, 1::2].bitcast(F32), 1.0)

            for _ in range(repeat):
                # ---- K+U projections (KT2/MU feed every attention tile) ----
                for q in range(nqc):
                    sl = slice(q * qc_w, (q + 1) * qc_w)
                    ps = psum_p.tile([HD + GQ, qc_w], F32, tag="pp")
                    for t in range(ht):
                        nc.tensor.matmul(
                            ps[:, :], lhsT=wku_sb[:, t, :],
                            rhs=xT_sb[:, t, sl],
                            start=(t == 0), stop=(t == ht - 1))
                    nc.vector.tensor_copy(KT2[0:HD, sl], ps[0:HD, :])
                    nc.vector.tensor_copy(UTst[HD:HD + GQ, sl], ps[HD:HD + GQ, :])
                    # duplicate this KT slice into partitions 64-127
                    nc.sync.dma_start(out=KT2[HD:P, sl], in_=KT2[0:HD, sl])
                # scatter U^T into MU u-columns via a DRAM bounce:
                # SBUF reads can't re-partition the free axis, DRAM APs can.
                nc.sync.dma_start(out=uscr[:, :], in_=UTst[HD:HD + GQ, :])
                for j in range(GQ):
                    nc.sync.dma_start(
                        out=MU[:, :, 2 * j].bitcast(F32),
                        in_=uscr[j, :].rearrange("(t p) -> p t", p=P))

                # ---- attention: Q-projection per chunk, then the pair loop ----
                for p2 in range(2):
                    h0, h1 = 2 * p2, 2 * p2 + 1
                    for q in range(nqc):
                        qsl = slice(q * qc_w, (q + 1) * qc_w)
                        psq = psum_p.tile([P, qc_w], F32, tag="pp")
                        for t in range(ht):
                            nc.tensor.matmul(
                                psq[:, :],
                                lhsT=wq_sb[:, t, p2 * P:(p2 + 1) * P],
                                rhs=xT_sb[:, t, qsl],
                                start=(t == 0), stop=(t == ht - 1))
                        nc.vector.tensor_copy(QT[:, p2, qsl], psq[:, :])

                        po0 = psum_o.tile([2, qc_w], F32, tag="po0")
                        po1 = psum_o.tile([2, qc_w], F32, tag="po1")
                        prev = None
                        for k in range(st):
                            ksl = slice(k * P, (k + 1) * P)
                            # one 2-bank psum tile holds both heads' scores
                            ps = psum_s.tile([P, 2, qc_w], F32, tag="ps")
                            # lhsT/rhs base partitions 0/64 -> PE row groups
                            # (0,0) and (64,0): the two matmuls run concurrently
                            nc.tensor.matmul(
                                ps[:, 0, :], lhsT=KT2[0:HD, ksl],
                                rhs=QT[0:HD, p2, qsl], start=True, stop=True,
                                tile_position=(0, 0))
                            nc.tensor.matmul(
                                ps[:, 1, :], lhsT=KT2[HD:P, ksl],
                                rhs=QT[HD:P, p2, qsl], start=True, stop=True,
                                tile_position=(HD, 0))
                            # one ACT call exps both heads' scores
                            e = epool.tile([P, 2, qc_w], F32R, tag="e")
                            nc.scalar.activation(
                                e[:, :, :], ps[:, :, :], AF.Exp,
                                bias=lm_sb[:, k:k + 1], scale=1.0 / np.sqrt(HD))
                            if prev is not None:
                                pk, pe = prev
                                nc.tensor.matmul(
                                    po0[:, :], lhsT=MU[:, pk, 2 * h0:2 * h0 + 2],
                                    rhs=pe[:, 0, :], start=(pk == 0), stop=False)
                                nc.tensor.matmul(
                                    po1[:, :], lhsT=MU[:, pk, 2 * h1:2 * h1 + 2],
                                    rhs=pe[:, 1, :], start=(pk == 0), stop=False)
                            prev = (k, e)
                        pk, pe = prev
                        nc.tensor.matmul(
                            po0[:, :], lhsT=MU[:, pk, 2 * h0:2 * h0 + 2],
                            rhs=pe[:, 0, :], start=(pk == 0), stop=True)
                        nc.tensor.matmul(
                            po1[:, :], lhsT=MU[:, pk, 2 * h1:2 * h1 + 2],
                            rhs=pe[:, 1, :], start=(pk == 0), stop=True)
                        ob0 = pcopy.tile([2, qc_w], F32, tag="ob0")
                        ob1 = pcopy.tile([2, qc_w], F32, tag="ob1")
                        nc.vector.tensor_copy(ob0[:, :], po0[:, :])
                        nc.vector.tensor_copy(ob1[:, :], po1[:, :])
                        nc.sync.dma_start(out=out[h0, :, qsl], in_=ob0[:, :])
                        nc.sync.dma_start(out=out[h1, :, qsl], in_=ob1[:, :])

    _split_excess_waits(nc)
    return nc


_NC_CACHE = {}


def _get_nc(s=S, repeat=1):
    key = (s, repeat)
    if key not in _NC_CACHE:
        _NC_CACHE[key] = build_nc(s, repeat)
    return _NC_CACHE[key]


def _round_f32r(a):
    """Round fp32 to fp32r (1+8+11 bits kept, RNE on the low 12 bits)."""
    u = np.ascontiguousarray(a, dtype=np.float32).view(np.uint32)
    r = (u + np.uint32(0x7FF) + ((u >> np.uint32(12)) & np.uint32(1))) & np.uint32(
        0xFFFFF000)
    return r.view(np.float32)


def make_inputs(hidden_states, attention_mask, wq, wk, wv, wo, s=S):
    """Host-side shard prep: per-core input dicts."""
    hidden_states = np.asarray(hidden_states, dtype=np.float32)
    attention_mask = np.asarray(attention_mask)
    wq = np.asarray(wq, dtype=np.float32)
    wk = np.asarray(wk, dtype=np.float32)
    wv = np.asarray(wv, dtype=np.float32)
    wo = np.asarray(wo, dtype=np.float32)
    st = s // P
    in_maps = []
    for c in range(NCORES):
        b, g = divmod(c, NKV)
        xT = np.ascontiguousarray(hidden_states[b, :s, :].T)
        wq_g = np.ascontiguousarray(wq[:, g * EW:(g + 1) * EW])
        wk_g = np.ascontiguousarray(wk[:, g * HD:(g + 1) * HD])
        wo_g = wo[g * EW:(g + 1) * EW, 0].reshape(GQ, HD).T  # [HD, GQ]
        wu_g = np.ascontiguousarray(wv[:, g * HD:(g + 1) * HD] @ wo_g)
        lm = np.where(attention_mask[b, :s] == 0, np.float32(-1e30),
                      np.float32(0.0)).astype(np.float32)
        lmT = np.ascontiguousarray(lm.reshape(st, P).T)
        in_maps.append({"xT": _round_f32r(xT), "wq": _round_f32r(wq_g),
                        "wk": _round_f32r(wk_g), "wu": _round_f32r(wu_g),
                        "lm": lmT})
    return in_maps


def combine(results, s=S):
    """Host-side gather: out[b,q] = sum over group cores and heads num/den."""
    out = np.zeros((B, s, 1), dtype=np.float32)
    for c in range(NCORES):
        b = c // NKV
        nd = results[c]["out"]          # [GQ, 2, s]
        out[b, :, 0] += (nd[:, 0, :] / nd[:, 1, :]).sum(axis=0)
    return out


def kernel(hidden_states, attention_mask, wq, wk, wv, wo):
    nc = _get_nc()
    in_maps = make_inputs(hidden_states, attention_mask, wq, wk, wv, wo)
    res = run_bass_kernel_spmd(nc, in_maps, core_ids=list(range(NCORES)))
    return combine(res.results)

